# revision 1
# baseline (speedup 1.0000x reference)
"""Causal multi-head self-attention (RoPE on input) for Trainium2, 8 NeuronCores.

Sharding: core c handles batch b = c//2 and head-group g = c%2 (8 of 16 heads).
Wq/Wk/Wv are split column-wise per head-group, Wo row-wise; each core produces a
partial (T, E) output and the host sums the two head-group partials per batch
and adds the bias.

Device layout notes:
- Activations are kept transposed (feature dim on partitions) so every matmul
  contraction runs over the partition dim with no on-device transposes.
- The input is passed de-interleaved (even RoPE pair lanes then odd lanes) so
  the RoPE pair swap is partition-aligned; the Wq/Wk/Wv rows carry the same
  permutation.
- Softmax is computed without max-subtraction (scores are O(+-10) for this
  distribution, exp is safe in fp32); the normalizer comes from a ones column
  appended to V, and the division is applied via gpsimd partition_broadcast.
"""

import numpy as np
import ml_dtypes

import concourse.bacc as bacc
import concourse.tile as tile
import concourse.mybir as mybir
from concourse import bass_utils
from concourse.bass_interp import get_hw_module

bf16 = ml_dtypes.bfloat16
BF = mybir.dt.bfloat16
F32 = mybir.dt.float32
EXP = mybir.ActivationFunctionType.Exp

B, T, E = 4, 2048, 1024
H, HD = 16, 64
G = 2  # head groups (tensor-parallel dimension)
HL = H // G  # heads per core
DL = HL * HD  # 512 local feature dim
P = 128
NT = T // P  # 16 tk tiles
NQ = T // 512  # 4 tq tiles
EC = E // P  # 8 contraction chunks over E
DC = DL // P  # 4 chunks over local head dims

_CACHE = {}
LAST_RESULT = None


def _build():
    nc = bacc.Bacc("TRN2", target_bir_lowering=False, debug=False, num_devices=8)
    xt_d = nc.dram_tensor("xt", (EC, P, T), BF, kind="ExternalInput").ap()
    sinh_d = nc.dram_tensor("sinh", (4, P, T), BF, kind="ExternalInput").ap()
    cosh_d = nc.dram_tensor("cosh", (4, P, T), BF, kind="ExternalInput").ap()
    wq_d = nc.dram_tensor("wq", (EC, P, DL), BF, kind="ExternalInput").ap()
    wk_d = nc.dram_tensor("wk", (EC, P, DL), BF, kind="ExternalInput").ap()
    wv_d = nc.dram_tensor("wv", (EC, P, DL), BF, kind="ExternalInput").ap()
    wo_d = nc.dram_tensor("wo", (DC, P, E), BF, kind="ExternalInput").ap()
    masks_d = nc.dram_tensor("masks", (4, P, 512), BF, kind="ExternalInput").ap()
    out_d = nc.dram_tensor("out", (T, E), F32, kind="ExternalOutput").ap()

    with tile.TileContext(nc) as tc:
        with tc.tile_pool(name="persist", bufs=1) as persist:
            rx = persist.tile([P, EC, T], BF)
            qT = persist.tile([P, DC, T], BF)
            kT = persist.tile([P, DC, T], BF)
            v = persist.tile([P, NT, HL, HD + 1], BF)
            oc = persist.tile([P, DC, T], BF)
            wq = persist.tile([P, EC, DL], BF)
            wk = persist.tile([P, EC, DL], BF)
            wv = persist.tile([P, EC, DL], BF)
            nc.sync.dma_start(wv, wv_d.rearrange("o p n -> p o n"))
            nc.sync.dma_start(wq, wq_d.rearrange("o p n -> p o n"))
            nc.sync.dma_start(wk, wk_d.rearrange("o p n -> p o n"))
            nc.vector.memset(v[:, :, :, HD : HD + 1], 1.0)

            # ---- Phase 1: RoPE + Q/K/V projections ----
            with (
                tc.tile_pool(name="xtp", bufs=1) as xtp,
                tc.tile_pool(name="tabs", bufs=2) as tabs,
                tc.tile_pool(name="tmps", bufs=2) as tmps,
                tc.tile_pool(name="mm1", bufs=4, space="PSUM") as mm1,
            ):
                xt = xtp.tile([P, EC, T], BF)
                nc.sync.dma_start(xt, xt_d.rearrange("o p t -> p o t"))

                # V projection (only needs xt)
                for tk in range(NT):
                    vp = mm1.tile([P, DL], F32, tag="mmp")
                    for j in range(EC):
                        nc.tensor.matmul(
                            vp,
                            lhsT=xt[:, j, P * tk : P * (tk + 1)],
                            rhs=wv[:, j, :],
                            start=(j == 0),
                            stop=(j == EC - 1),
                        )
                    nc.vector.tensor_copy(
                        v[:, tk, :, 0:HD],
                        vp.rearrange("p (h d) -> p h d", h=HL),
                    )

                # RoPE: rx = x*cos +/- swap(x)*sin, pair lanes de-interleaved
                for u in range(4):
                    sin_t = tabs.tile([P, T], BF, tag="sin")
                    nc.sync.dma_start(sin_t, sinh_d[u])
                    cos_t = tabs.tile([P, T], BF, tag="cos")
                    nc.sync.dma_start(cos_t, cosh_d[u])
                    xe = xt[:, u, :]
                    xo = xt[:, u + 4, :]
                    t1 = tmps.tile([P, T], BF, tag="t1")
                    nc.vector.tensor_mul(t1, xe, cos_t)
                    t2 = tmps.tile([P, T], BF, tag="t2")
                    nc.vector.tensor_mul(t2, xo, sin_t)
                    nc.vector.tensor_sub(rx[:, u, :], t1, t2)
                    t3 = tmps.tile([P, T], BF, tag="t1")
                    nc.vector.tensor_mul(t3, xo, cos_t)
                    t4 = tmps.tile([P, T], BF, tag="t2")
                    nc.vector.tensor_mul(t4, xe, sin_t)
                    nc.vector.tensor_add(rx[:, u + 4, :], t3, t4)

                # Q/K projections from rx
                for w_sb, dst in ((wq, qT), (wk, kT)):
                    for dt_ in range(DC):
                        for ti in range(NQ):
                            pp = mm1.tile([P, 512], F32, tag="mmp")
                            for j in range(EC):
                                nc.tensor.matmul(
                                    pp,
                                    lhsT=w_sb[:, j, P * dt_ : P * (dt_ + 1)],
                                    rhs=rx[:, j, 512 * ti : 512 * (ti + 1)],
                                    start=(j == 0),
                                    stop=(j == EC - 1),
                                )
                            nc.scalar.copy(dst[:, dt_, 512 * ti : 512 * (ti + 1)], pp)

            # ---- Phase 2: attention per (head, tq-tile) ----
            with (
                tc.tile_pool(name="mask", bufs=1) as mpool,
                tc.tile_pool(name="att", bufs=8) as apool,
                tc.tile_pool(name="norm", bufs=4) as npool,
                tc.tile_pool(name="sps", bufs=4, space="PSUM") as spool,
                tc.tile_pool(name="ops", bufs=2, space="PSUM") as opool,
            ):
                masks = mpool.tile([P, 4, 512], BF)
                nc.sync.dma_start(masks, masks_d.rearrange("r p n -> p r n"))

                for h in range(HL):
                    pb = HD * (h % 2)
                    dt_ = h // 2
                    for i in range(NQ):
                        nj = 4 * i + 4
                        op = opool.tile([HD + 1, 512], F32, tag="o")
                        at_prev = None
                        for j in range(nj):
                            sp = spool.tile([P, 512], F32, tag="s")
                            nc.tensor.matmul(
                                sp,
                                lhsT=kT[pb : pb + HD, dt_, P * j : P * (j + 1)],
                                rhs=qT[pb : pb + HD, dt_, 512 * i : 512 * (i + 1)],
                                start=True,
                                stop=True,
                            )
                            at = apool.tile([P, 512], BF, tag="a")
                            nc.scalar.activation(at, sp, EXP, scale=0.125)
                            if j >= 4 * i:
                                nc.vector.tensor_mul(at, at, masks[:, j - 4 * i, :])
                            if at_prev is not None:
                                nc.tensor.matmul(
                                    op,
                                    lhsT=v[:, j - 1, h, :],
                                    rhs=at_prev,
                                    start=(j == 1),
                                    stop=False,
                                )
                            at_prev = at
                        nc.tensor.matmul(
                            op,
                            lhsT=v[:, nj - 1, h, :],
                            rhs=at_prev,
                            start=False,
                            stop=True,
                        )
                        rc = npool.tile([1, 512], F32, tag="rc")
                        nc.vector.reciprocal(rc, op[HD : HD + 1, :])
                        bc = npool.tile([HD, 512], F32, tag="bc")
                        nc.gpsimd.partition_broadcast(bc, rc)
                        nc.vector.tensor_mul(
                            oc[pb : pb + HD, dt_, 512 * i : 512 * (i + 1)],
                            op[0:HD, :],
                            bc,
                        )

            # ---- Phase 3: output projection ----
            with (
                tc.tile_pool(name="wop", bufs=1) as wop,
                tc.tile_pool(name="oout", bufs=3) as oop,
                tc.tile_pool(name="mm3", bufs=4, space="PSUM") as mm3,
            ):
                wo = wop.tile([P, DC, E], BF)
                nc.sync.dma_start(wo, wo_d.rearrange("o p n -> p o n"))
                for tt in range(NT):
                    ot = oop.tile([P, E], F32, tag="ot")
                    for et in range(2):
                        pp = mm3.tile([P, 512], F32, tag="p3")
                        for kk in range(DC):
                            nc.tensor.matmul(
                                pp,
                                lhsT=oc[:, kk, P * tt : P * (tt + 1)],
                                rhs=wo[:, kk, 512 * et : 512 * (et + 1)],
                                start=(kk == 0),
                                stop=(kk == DC - 1),
                            )
                        nc.scalar.copy(ot[:, 512 * et : 512 * (et + 1)], pp)
                    nc.sync.dma_start(out_d[P * tt : P * (tt + 1), :], ot)

    nc.compile()
    nc.m = get_hw_module(nc.m)
    return nc


def _prep_inputs(input, Wq, Wk, Wv, Wo):
    """Host-side shard prep: transpose/de-interleave/cast. Returns 8 in_maps."""
    perm = np.concatenate([np.arange(0, E, 2), np.arange(1, E, 2)])

    u = np.arange(E // 2, dtype=np.float64)
    thetas = 10000.0 ** (-2.0 * u / E)
    ang = np.arange(T, dtype=np.float64)[:, None] * thetas[None, :]
    sinh = np.sin(ang).T.reshape(4, P, T).astype(bf16)
    cosh = np.cos(ang).T.reshape(4, P, T).astype(bf16)

    masks = np.zeros((4, P, 512), np.float32)
    f = np.arange(512)
    for r in range(4):
        for p in range(P):
            masks[r, p] = (f >= P * r + p).astype(np.float32)
    masks = masks.astype(bf16)

    xt = [
        np.ascontiguousarray(input[b].T[perm]).reshape(EC, P, T).astype(bf16)
        for b in range(B)
    ]
    WqT, WkT, WvT = Wq.T[perm], Wk.T[perm], Wv.T[perm]
    wq_g = [
        np.ascontiguousarray(WqT[:, DL * g : DL * (g + 1)])
        .reshape(EC, P, DL)
        .astype(bf16)
        for g in range(G)
    ]
    wk_g = [
        np.ascontiguousarray(WkT[:, DL * g : DL * (g + 1)])
        .reshape(EC, P, DL)
        .astype(bf16)
        for g in range(G)
    ]
    wv_g = [
        np.ascontiguousarray(WvT[:, DL * g : DL * (g + 1)])
        .reshape(EC, P, DL)
        .astype(bf16)
        for g in range(G)
    ]
    wo_g = [
        np.ascontiguousarray(Wo.T[DL * g : DL * (g + 1)])
        .reshape(DC, P, E)
        .astype(bf16)
        for g in range(G)
    ]

    in_maps = []
    for c in range(8):
        b, g = c // 2, c % 2
        in_maps.append(
            {
                "xt": xt[b],
                "sinh": sinh,
                "cosh": cosh,
                "wq": wq_g[g],
                "wk": wk_g[g],
                "wv": wv_g[g],
                "wo": wo_g[g],
                "masks": masks,
            }
        )
    return in_maps


def kernel(input, Wq, Wk, Wv, Wo, bo):
    global LAST_RESULT
    input = np.asarray(input, np.float32)
    Wq, Wk, Wv, Wo = (np.asarray(w, np.float32) for w in (Wq, Wk, Wv, Wo))
    bo = np.asarray(bo, np.float32)

    if "nc" not in _CACHE:
        _CACHE["nc"] = _build()
    nc = _CACHE["nc"]

    in_maps = _prep_inputs(input, Wq, Wk, Wv, Wo)
    res = bass_utils.run_bass_kernel_spmd(nc, in_maps, core_ids=list(range(8)))
    LAST_RESULT = res

    out = np.empty((B, T, E), np.float32)
    for b in range(B):
        out[b] = res.results[2 * b]["out"] + res.results[2 * b + 1]["out"] + bo
    return out


# revision 13
# speedup vs baseline: 1.4710x; 1.4710x over previous
"""Causal multi-head self-attention (RoPE on input) for Trainium2, 8 NeuronCores.

Sharding: core c handles batch b = c//2 and head-group g = c%2 (8 of 16 heads).
Wq/Wk/Wv are split column-wise per head-group, Wo row-wise; each core produces a
partial (T, E) output and the host sums the two head-group partials per batch
and adds the bias.

Device layout notes:
- Activations are kept transposed (feature dim on partitions) so every matmul
  contraction runs over the partition dim with no on-device transposes.
- The input is passed de-interleaved (even RoPE pair lanes then odd lanes) so
  the RoPE pair swap is partition-aligned; the Wq/Wk/Wv rows carry the same
  permutation.
- Softmax is computed without max-subtraction (scores are O(+-10) for this
  distribution, exp is safe in fp32); the normalizer comes from a ones column
  appended to V, and the division is applied via gpsimd partition_broadcast.
"""

import numpy as np
import ml_dtypes

import concourse.bacc as bacc
import concourse.tile as tile
import concourse.mybir as mybir
from concourse import bass_utils
from concourse.bass_interp import get_hw_module

bf16 = ml_dtypes.bfloat16
BF = mybir.dt.bfloat16
F32 = mybir.dt.float32
EXP = mybir.ActivationFunctionType.Exp

B, T, E = 4, 2048, 1024
H, HD = 16, 64
G = 2  # head groups (tensor-parallel dimension)
HL = H // G  # heads per core
DL = HL * HD  # 512 local feature dim
P = 128
NT = T // P  # 16 tk tiles
NQ = T // 512  # 4 tq tiles
EC = E // P  # 8 contraction chunks over E
DC = DL // P  # 4 chunks over local head dims

_CACHE = {}
LAST_RESULT = None


def _build():
    nc = bacc.Bacc("TRN2", target_bir_lowering=False, debug=False, num_devices=8)
    xt_d = nc.dram_tensor("xt", (EC, P, T), BF, kind="ExternalInput").ap()
    sinh_d = nc.dram_tensor("sinh", (4, P, T), BF, kind="ExternalInput").ap()
    cosh_d = nc.dram_tensor("cosh", (4, P, T), BF, kind="ExternalInput").ap()
    wq_d = nc.dram_tensor("wq", (EC, P, DL), BF, kind="ExternalInput").ap()
    wk_d = nc.dram_tensor("wk", (EC, P, DL), BF, kind="ExternalInput").ap()
    wv_d = nc.dram_tensor("wv", (EC, P, DL), BF, kind="ExternalInput").ap()
    wo_d = nc.dram_tensor("wo", (DC, P, E), BF, kind="ExternalInput").ap()
    masks_d = nc.dram_tensor("masks", (4, P, 512), BF, kind="ExternalInput").ap()
    out_d = nc.dram_tensor("out", (T, E), F32, kind="ExternalOutput").ap()

    with tile.TileContext(nc) as tc:
        with tc.tile_pool(name="persist", bufs=1) as persist:
            rx = persist.tile([P, EC, T], BF)
            qT = persist.tile([P, DC, T], BF)
            kT = persist.tile([P, DC, T], BF)
            v = persist.tile([P, NT, HL, HD + 1], BF)
            oc = persist.tile([P, DC, T], BF)
            wq = persist.tile([P, EC, DL], BF)
            wk = persist.tile([P, EC, DL], BF)
            wv = persist.tile([P, EC, DL], BF)
            nc.sync.dma_start(wv, wv_d.rearrange("o p n -> p o n"))
            nc.sync.dma_start(wq, wq_d.rearrange("o p n -> p o n"))
            nc.sync.dma_start(wk, wk_d.rearrange("o p n -> p o n"))
            nc.vector.memset(v[:, :, :, HD : HD + 1], 1.0)

            # ---- Phase 1: RoPE + Q/K/V projections ----
            with (
                tc.tile_pool(name="xtp", bufs=1) as xtp,
                tc.tile_pool(name="tabs", bufs=2) as tabs,
                tc.tile_pool(name="tmps", bufs=2) as tmps,
                tc.tile_pool(name="mm1", bufs=4, space="PSUM") as mm1,
            ):
                xt = xtp.tile([P, EC, T], BF)
                nc.sync.dma_start(xt, xt_d.rearrange("o p t -> p o t"))

                # V projection (only needs xt)
                for tk in range(NT):
                    vp = mm1.tile([P, DL], F32, tag="mmp")
                    for j in range(EC):
                        nc.tensor.matmul(
                            vp,
                            lhsT=xt[:, j, P * tk : P * (tk + 1)],
                            rhs=wv[:, j, :],
                            start=(j == 0),
                            stop=(j == EC - 1),
                        )
                    nc.vector.tensor_copy(
                        v[:, tk, :, 0:HD],
                        vp.rearrange("p (h d) -> p h d", h=HL),
                    )

                # RoPE: rx = x*cos +/- swap(x)*sin, pair lanes de-interleaved
                for u in range(4):
                    sin_t = tabs.tile([P, T], BF, tag="sin")
                    nc.sync.dma_start(sin_t, sinh_d[u])
                    cos_t = tabs.tile([P, T], BF, tag="cos")
                    nc.sync.dma_start(cos_t, cosh_d[u])
                    xe = xt[:, u, :]
                    xo = xt[:, u + 4, :]
                    t1 = tmps.tile([P, T], BF, tag="t1")
                    nc.vector.tensor_mul(t1, xe, cos_t)
                    t2 = tmps.tile([P, T], BF, tag="t2")
                    nc.vector.tensor_mul(t2, xo, sin_t)
                    nc.vector.tensor_sub(rx[:, u, :], t1, t2)
                    t3 = tmps.tile([P, T], BF, tag="t1")
                    nc.vector.tensor_mul(t3, xo, cos_t)
                    t4 = tmps.tile([P, T], BF, tag="t2")
                    nc.vector.tensor_mul(t4, xe, sin_t)
                    nc.vector.tensor_add(rx[:, u + 4, :], t3, t4)

                # Q/K projections from rx
                for w_sb, dst in ((wq, qT), (wk, kT)):
                    for dt_ in range(DC):
                        for ti in range(NQ):
                            pp = mm1.tile([P, 512], F32, tag="mmp")
                            for j in range(EC):
                                nc.tensor.matmul(
                                    pp,
                                    lhsT=w_sb[:, j, P * dt_ : P * (dt_ + 1)],
                                    rhs=rx[:, j, 512 * ti : 512 * (ti + 1)],
                                    start=(j == 0),
                                    stop=(j == EC - 1),
                                )
                            nc.scalar.copy(dst[:, dt_, 512 * ti : 512 * (ti + 1)], pp)

            # ---- Phase 2: attention, head pairs share the PE array via row
            # tiling (head 2p on array rows 0-63, head 2p+1 on rows 64-127)
            # and share one (128, 1024) exp per tk-tile. PV matmuls trail the
            # scores pipeline by 2 tk-tiles so the PE never waits on the
            # Scalar engine's exp.
            with (
                tc.tile_pool(name="mask", bufs=1) as mpool,
                tc.tile_pool(name="att", bufs=6) as apool,
                tc.tile_pool(name="norm", bufs=2) as npool,
                tc.tile_pool(name="sps", bufs=3, space="PSUM") as spool,
                tc.tile_pool(name="ops", bufs=2, space="PSUM") as opool,
                tc.tile_pool(name="dramn", bufs=1, space="DRAM") as dpool,
            ):
                masks = mpool.tile([P, 4, 512], BF)
                nc.sync.dma_start(masks, masks_d.rearrange("r p n -> p r n"))
                # row-sum strips from all cells of head-pairs {2g2, 2g2+1} are
                # DMA-packed into packed[g2] so one cheap (16, 512) reciprocal
                # covers 16 cells; the result is DMA-broadcast back across the
                # 64 head-dim partitions.
                packed = [
                    npool.tile(
                        [16, 512], F32, tag=f"packed{g2}", name=f"packed{g2}"
                    )
                    for g2 in range(2)
                ]
                rpk = [
                    npool.tile([16, 512], F32, tag=f"rpk{g2}", name=f"rpk{g2}")
                    for g2 in range(2)
                ]
                rpk_d = dpool.tile([2, 16, 512], F32)

                for hp in range(HL // 2):
                    h0, h1 = 2 * hp, 2 * hp + 1
                    g2, hp2 = hp // 2, hp % 2
                    for i in range(NQ):
                        nj = 4 * i + 4
                        tq = slice(512 * i, 512 * (i + 1))
                        op0 = opool.tile([HD + 1, 512], F32, tag="o")
                        op1 = opool.tile([HD + 1, 512], F32, tag="o")
                        ats = []

                        def emit_pv(jp, at_jp):
                            nc.tensor.matmul(
                                op0,
                                lhsT=v[:, jp, h0, :],
                                rhs=at_jp[:, 0, :],
                                start=(jp == 0),
                                stop=(jp == nj - 1),
                            )
                            nc.tensor.matmul(
                                op1,
                                lhsT=v[:, jp, h1, :],
                                rhs=at_jp[:, 1, :],
                                start=(jp == 0),
                                stop=(jp == nj - 1),
                            )

                        for j in range(nj):
                            sp = spool.tile([P, 2, 512], F32, tag="s")
                            nc.tensor.matmul(
                                sp[:, 0, :],
                                lhsT=kT[0:HD, hp, P * j : P * (j + 1)],
                                rhs=qT[0:HD, hp, tq],
                                start=True,
                                stop=True,
                            )
                            nc.tensor.matmul(
                                sp[:, 1, :],
                                lhsT=kT[HD:P, hp, P * j : P * (j + 1)],
                                rhs=qT[HD:P, hp, tq],
                                start=True,
                                stop=True,
                            )
                            at = apool.tile([P, 2, 512], BF, tag="a")
                            nc.scalar.activation(at, sp, EXP, scale=0.125)
                            if j >= 4 * i:
                                nc.vector.tensor_mul(
                                    at,
                                    at,
                                    masks[:, j - 4 * i, None, :].to_broadcast(
                                        (P, 2, 512)
                                    ),
                                )
                            ats.append(at)
                            if j >= 2:
                                emit_pv(j - 2, ats[j - 2])
                        for jp in range(max(nj - 2, 0), nj):
                            emit_pv(jp, ats[jp])

                        r0, r1 = 8 * hp2 + i, 8 * hp2 + 4 + i
                        s0 = npool.tile([1, 512], F32, tag="s0")
                        nc.vector.tensor_copy(s0, op0[HD : HD + 1, :])
                        nc.sync.dma_start(packed[g2][r0 : r0 + 1, :], s0)
                        s1 = npool.tile([1, 512], F32, tag="s1")
                        nc.vector.tensor_copy(s1, op1[HD : HD + 1, :])
                        nc.sync.dma_start(packed[g2][r1 : r1 + 1, :], s1)
                        nc.vector.tensor_copy(oc[0:HD, hp, tq], op0[0:HD, :])
                        nc.vector.tensor_copy(oc[HD:P, hp, tq], op1[0:HD, :])

                    if hp2 == 1:
                        nc.vector.reciprocal(rpk[g2], packed[g2])
                        nc.sync.dma_start(rpk_d[g2], rpk[g2])

                # broadcast 1/sum across the 64 head-dim partitions (DMA with
                # partition-step-0 DRAM source), then normalize in place
                for hp in range(HL // 2):
                    g2, hp2 = hp // 2, hp % 2
                    rb = npool.tile([P, T], F32, tag="rb")
                    for i in range(NQ):
                        tq = slice(512 * i, 512 * (i + 1))
                        r0, r1 = 8 * hp2 + i, 8 * hp2 + 4 + i
                        nc.sync.dma_start(
                            rb[0:HD, tq],
                            rpk_d[g2, r0 : r0 + 1, :].to_broadcast((HD, 512)),
                        )
                        nc.sync.dma_start(
                            rb[HD:P, tq],
                            rpk_d[g2, r1 : r1 + 1, :].to_broadcast((HD, 512)),
                        )
                    nc.vector.tensor_mul(oc[:, hp, :], oc[:, hp, :], rb)

            # ---- Phase 3: output projection ----
            with (
                tc.tile_pool(name="wop", bufs=1) as wop,
                tc.tile_pool(name="oout", bufs=3) as oop,
                tc.tile_pool(name="mm3", bufs=4, space="PSUM") as mm3,
            ):
                wo = wop.tile([P, DC, E], BF)
                nc.sync.dma_start(wo, wo_d.rearrange("o p n -> p o n"))
                for tt in range(NT):
                    ot = oop.tile([P, E], F32, tag="ot")
                    for et in range(2):
                        pp = mm3.tile([P, 512], F32, tag="p3")
                        for kk in range(DC):
                            nc.tensor.matmul(
                                pp,
                                lhsT=oc[:, kk, P * tt : P * (tt + 1)],
                                rhs=wo[:, kk, 512 * et : 512 * (et + 1)],
                                start=(kk == 0),
                                stop=(kk == DC - 1),
                            )
                        nc.scalar.copy(ot[:, 512 * et : 512 * (et + 1)], pp)
                    nc.sync.dma_start(out_d[P * tt : P * (tt + 1), :], ot)

    nc.compile()
    nc.m = get_hw_module(nc.m)
    return nc


def _prep_inputs(input, Wq, Wk, Wv, Wo):
    """Host-side shard prep: transpose/de-interleave/cast. Returns 8 in_maps."""
    perm = np.concatenate([np.arange(0, E, 2), np.arange(1, E, 2)])

    u = np.arange(E // 2, dtype=np.float64)
    thetas = 10000.0 ** (-2.0 * u / E)
    ang = np.arange(T, dtype=np.float64)[:, None] * thetas[None, :]
    sinh = np.sin(ang).T.reshape(4, P, T).astype(bf16)
    cosh = np.cos(ang).T.reshape(4, P, T).astype(bf16)

    masks = np.zeros((4, P, 512), np.float32)
    f = np.arange(512)
    for r in range(4):
        for p in range(P):
            masks[r, p] = (f >= P * r + p).astype(np.float32)
    masks = masks.astype(bf16)

    xt = [
        np.ascontiguousarray(input[b].T[perm]).reshape(EC, P, T).astype(bf16)
        for b in range(B)
    ]
    WqT, WkT, WvT = Wq.T[perm], Wk.T[perm], Wv.T[perm]
    wq_g = [
        np.ascontiguousarray(WqT[:, DL * g : DL * (g + 1)])
        .reshape(EC, P, DL)
        .astype(bf16)
        for g in range(G)
    ]
    wk_g = [
        np.ascontiguousarray(WkT[:, DL * g : DL * (g + 1)])
        .reshape(EC, P, DL)
        .astype(bf16)
        for g in range(G)
    ]
    wv_g = [
        np.ascontiguousarray(WvT[:, DL * g : DL * (g + 1)])
        .reshape(EC, P, DL)
        .astype(bf16)
        for g in range(G)
    ]
    wo_g = [
        np.ascontiguousarray(Wo.T[DL * g : DL * (g + 1)])
        .reshape(DC, P, E)
        .astype(bf16)
        for g in range(G)
    ]

    in_maps = []
    for c in range(8):
        b, g = c // 2, c % 2
        in_maps.append(
            {
                "xt": xt[b],
                "sinh": sinh,
                "cosh": cosh,
                "wq": wq_g[g],
                "wk": wk_g[g],
                "wv": wv_g[g],
                "wo": wo_g[g],
                "masks": masks,
            }
        )
    return in_maps


def kernel(input, Wq, Wk, Wv, Wo, bo):
    global LAST_RESULT
    input = np.asarray(input, np.float32)
    Wq, Wk, Wv, Wo = (np.asarray(w, np.float32) for w in (Wq, Wk, Wv, Wo))
    bo = np.asarray(bo, np.float32)

    if "nc" not in _CACHE:
        _CACHE["nc"] = _build()
    nc = _CACHE["nc"]

    in_maps = _prep_inputs(input, Wq, Wk, Wv, Wo)
    res = bass_utils.run_bass_kernel_spmd(nc, in_maps, core_ids=list(range(8)))
    LAST_RESULT = res

    out = np.empty((B, T, E), np.float32)
    for b in range(B):
        out[b] = res.results[2 * b]["out"] + res.results[2 * b + 1]["out"] + bo
    return out


# revision 17
# speedup vs baseline: 1.4748x; 1.0026x over previous
"""Causal multi-head self-attention (RoPE on input) for Trainium2, 8 NeuronCores.

Sharding: core c handles batch b = c//2 and head-group g = c%2 (8 of 16 heads).
Wq/Wk/Wv are split column-wise per head-group, Wo row-wise; each core produces a
partial (T, E) output and the host sums the two head-group partials per batch
and adds the bias.

Device layout notes:
- Activations are kept transposed (feature dim on partitions) so every matmul
  contraction runs over the partition dim with no on-device transposes.
- The input is passed de-interleaved (even RoPE pair lanes then odd lanes) so
  the RoPE pair swap is partition-aligned; the Wq/Wk/Wv rows carry the same
  permutation.
- Softmax is computed without max-subtraction (scores are O(+-10) for this
  distribution, exp is safe in fp32); the normalizer comes from a ones column
  appended to V, and the division is applied via gpsimd partition_broadcast.
"""

import numpy as np
import ml_dtypes

import concourse.bacc as bacc
import concourse.tile as tile
import concourse.mybir as mybir
from concourse import bass_utils
from concourse.bass_interp import get_hw_module

bf16 = ml_dtypes.bfloat16
BF = mybir.dt.bfloat16
F32 = mybir.dt.float32
EXP = mybir.ActivationFunctionType.Exp

B, T, E = 4, 2048, 1024
H, HD = 16, 64
G = 2  # head groups (tensor-parallel dimension)
HL = H // G  # heads per core
DL = HL * HD  # 512 local feature dim
P = 128
NT = T // P  # 16 tk tiles
NQ = T // 512  # 4 tq tiles
EC = E // P  # 8 contraction chunks over E
DC = DL // P  # 4 chunks over local head dims

_CACHE = {}
LAST_RESULT = None


def _build():
    nc = bacc.Bacc("TRN2", target_bir_lowering=False, debug=False, num_devices=8)
    xt_d = nc.dram_tensor("xt", (EC, P, T), BF, kind="ExternalInput").ap()
    sinh_d = nc.dram_tensor("sinh", (4, P, T), BF, kind="ExternalInput").ap()
    cosh_d = nc.dram_tensor("cosh", (4, P, T), BF, kind="ExternalInput").ap()
    wq_d = nc.dram_tensor("wq", (EC, P, DL), BF, kind="ExternalInput").ap()
    wk_d = nc.dram_tensor("wk", (EC, P, DL), BF, kind="ExternalInput").ap()
    wv_d = nc.dram_tensor("wv", (EC, P, DL), BF, kind="ExternalInput").ap()
    wo_d = nc.dram_tensor("wo", (DC, P, E), BF, kind="ExternalInput").ap()
    masks_d = nc.dram_tensor("masks", (4, P, 512), BF, kind="ExternalInput").ap()
    out_d = nc.dram_tensor("out", (T, E), F32, kind="ExternalOutput").ap()

    with tile.TileContext(nc) as tc:
        with tc.tile_pool(name="persist", bufs=1) as persist:
            rx = persist.tile([P, EC, T], BF)
            qT = persist.tile([P, DC, T], BF)
            kT = persist.tile([P, DC, T], BF)
            v = persist.tile([P, NT, HL, HD + 1], BF)
            oc = persist.tile([P, DC, T], BF)
            wq = persist.tile([P, EC, DL], BF)
            wk = persist.tile([P, EC, DL], BF)
            wv = persist.tile([P, EC, DL], BF)
            nc.sync.dma_start(wv, wv_d.rearrange("o p n -> p o n"))
            nc.sync.dma_start(wq, wq_d.rearrange("o p n -> p o n"))
            nc.sync.dma_start(wk, wk_d.rearrange("o p n -> p o n"))
            nc.vector.memset(v[:, :, :, HD : HD + 1], 1.0)

            # ---- Phase 1: RoPE + Q/K/V projections ----
            with (
                tc.tile_pool(name="xtp", bufs=1) as xtp,
                tc.tile_pool(name="tabs", bufs=2) as tabs,
                tc.tile_pool(name="tmps", bufs=2) as tmps,
                tc.tile_pool(name="mm1", bufs=4, space="PSUM") as mm1,
            ):
                xt = xtp.tile([P, EC, T], BF)

                # RoPE: rx = x*cos +/- swap(x)*sin, pair lanes de-interleaved.
                # xt is loaded in (even, odd) chunk pairs so rope iteration u
                # starts as soon as its pair lands.
                for u in range(4):
                    nc.sync.dma_start(xt[:, u, :], xt_d[u].rearrange("p t -> p t"))
                    nc.sync.dma_start(xt[:, u + 4, :], xt_d[u + 4])
                    sin_t = tabs.tile([P, T], BF, tag="sin")
                    nc.sync.dma_start(sin_t, sinh_d[u])
                    cos_t = tabs.tile([P, T], BF, tag="cos")
                    nc.sync.dma_start(cos_t, cosh_d[u])
                    xe = xt[:, u, :]
                    xo = xt[:, u + 4, :]
                    t1 = tmps.tile([P, T], BF, tag="t1")
                    nc.vector.tensor_mul(t1, xe, cos_t)
                    t2 = tmps.tile([P, T], BF, tag="t2")
                    nc.vector.tensor_mul(t2, xo, sin_t)
                    nc.vector.tensor_sub(rx[:, u, :], t1, t2)
                    t3 = tmps.tile([P, T], BF, tag="t1")
                    nc.vector.tensor_mul(t3, xo, cos_t)
                    t4 = tmps.tile([P, T], BF, tag="t2")
                    nc.vector.tensor_mul(t4, xe, sin_t)
                    nc.vector.tensor_add(rx[:, u + 4, :], t3, t4)

                # V projection (only needs xt)
                for tk in range(NT):
                    vp = mm1.tile([P, DL], F32, tag="mmp")
                    for j in range(EC):
                        nc.tensor.matmul(
                            vp,
                            lhsT=xt[:, j, P * tk : P * (tk + 1)],
                            rhs=wv[:, j, :],
                            start=(j == 0),
                            stop=(j == EC - 1),
                        )
                    nc.vector.tensor_copy(
                        v[:, tk, :, 0:HD],
                        vp.rearrange("p (h d) -> p h d", h=HL),
                    )

                # Q/K projections from rx
                for w_sb, dst in ((wq, qT), (wk, kT)):
                    for dt_ in range(DC):
                        for ti in range(NQ):
                            pp = mm1.tile([P, 512], F32, tag="mmp")
                            for j in range(EC):
                                nc.tensor.matmul(
                                    pp,
                                    lhsT=w_sb[:, j, P * dt_ : P * (dt_ + 1)],
                                    rhs=rx[:, j, 512 * ti : 512 * (ti + 1)],
                                    start=(j == 0),
                                    stop=(j == EC - 1),
                                )
                            nc.scalar.copy(dst[:, dt_, 512 * ti : 512 * (ti + 1)], pp)

            # ---- Phase 2: attention, head pairs share the PE array via row
            # tiling (head 2p on array rows 0-63, head 2p+1 on rows 64-127)
            # and share one (128, 1024) exp per tk-tile. PV matmuls trail the
            # scores pipeline by 2 tk-tiles so the PE never waits on the
            # Scalar engine's exp.
            with (
                tc.tile_pool(name="mask", bufs=1) as mpool,
                tc.tile_pool(name="att", bufs=6) as apool,
                tc.tile_pool(name="norm", bufs=2) as npool,
                tc.tile_pool(name="sps", bufs=2, space="PSUM") as spool,
                tc.tile_pool(name="ops", bufs=4, space="PSUM") as opool,
                tc.tile_pool(name="dramn", bufs=1, space="DRAM") as dpool,
            ):
                masks = mpool.tile([P, 4, 512], BF)
                nc.sync.dma_start(masks, masks_d.rearrange("r p n -> p r n"))
                # row-sum strips from the 8 cells of a head-pair are
                # DMA-packed into packed[hp] so one cheap (8, 512) reciprocal
                # covers them; the result is DMA-broadcast back across the
                # 64 head-dim partitions (via a DRAM bounce, which permits a
                # partition-step-0 source) while the next pair's attention
                # runs.
                rpk_d = dpool.tile([HL // 2, 8, 512], F32)

                for hp in range(HL // 2):
                    h0, h1 = 2 * hp, 2 * hp + 1
                    packed = npool.tile(
                        [8, 512], F32, tag="packed", name=f"packed{hp}"
                    )
                    for i in range(NQ):
                        nj = 4 * i + 4
                        tq = slice(512 * i, 512 * (i + 1))
                        op0 = opool.tile([HD + 1, 512], F32, tag="o")
                        op1 = opool.tile([HD + 1, 512], F32, tag="o")
                        ats = []

                        def emit_pv(jp, at_jp):
                            nc.tensor.matmul(
                                op0,
                                lhsT=v[:, jp, h0, :],
                                rhs=at_jp[:, 0, :],
                                start=(jp == 0),
                                stop=(jp == nj - 1),
                            )
                            nc.tensor.matmul(
                                op1,
                                lhsT=v[:, jp, h1, :],
                                rhs=at_jp[:, 1, :],
                                start=(jp == 0),
                                stop=(jp == nj - 1),
                            )

                        for j in range(nj):
                            sp = spool.tile([P, 2, 512], F32, tag="s")
                            nc.tensor.matmul(
                                sp[:, 0, :],
                                lhsT=kT[0:HD, hp, P * j : P * (j + 1)],
                                rhs=qT[0:HD, hp, tq],
                                start=True,
                                stop=True,
                            )
                            nc.tensor.matmul(
                                sp[:, 1, :],
                                lhsT=kT[HD:P, hp, P * j : P * (j + 1)],
                                rhs=qT[HD:P, hp, tq],
                                start=True,
                                stop=True,
                            )
                            at = apool.tile([P, 2, 512], BF, tag="a")
                            nc.scalar.activation(at, sp, EXP, scale=0.125)
                            if j >= 4 * i:
                                nc.vector.tensor_mul(
                                    at,
                                    at,
                                    masks[:, j - 4 * i, None, :].to_broadcast(
                                        (P, 2, 512)
                                    ),
                                )
                            ats.append(at)
                            if j >= 2:
                                emit_pv(j - 2, ats[j - 2])
                        for jp in range(max(nj - 2, 0), nj):
                            emit_pv(jp, ats[jp])

                        r0, r1 = i, 4 + i
                        s0 = npool.tile([1, 512], F32, tag="s0")
                        nc.vector.tensor_copy(s0, op0[HD : HD + 1, :])
                        nc.sync.dma_start(packed[r0 : r0 + 1, :], s0)
                        s1 = npool.tile([1, 512], F32, tag="s1")
                        nc.vector.tensor_copy(s1, op1[HD : HD + 1, :])
                        nc.sync.dma_start(packed[r1 : r1 + 1, :], s1)
                        nc.vector.tensor_copy(oc[0:HD, hp, tq], op0[0:HD, :])
                        nc.vector.tensor_copy(oc[HD:P, hp, tq], op1[0:HD, :])

                    # normalize this head-pair while the next pair's
                    # attention runs
                    rpk = npool.tile([8, 512], F32, tag="rpk", name=f"rpk{hp}")
                    nc.vector.reciprocal(rpk, packed)
                    nc.sync.dma_start(rpk_d[hp], rpk)
                    rb = npool.tile([P, T], F32, tag="rb")
                    for i in range(NQ):
                        tq = slice(512 * i, 512 * (i + 1))
                        nc.sync.dma_start(
                            rb[0:HD, tq],
                            rpk_d[hp, i : i + 1, :].to_broadcast((HD, 512)),
                        )
                        nc.sync.dma_start(
                            rb[HD:P, tq],
                            rpk_d[hp, 4 + i : 5 + i, :].to_broadcast((HD, 512)),
                        )
                    nc.vector.tensor_mul(oc[:, hp, :], oc[:, hp, :], rb)

            # ---- Phase 3: output projection ----
            with (
                tc.tile_pool(name="wop", bufs=1) as wop,
                tc.tile_pool(name="oout", bufs=3) as oop,
                tc.tile_pool(name="mm3", bufs=4, space="PSUM") as mm3,
            ):
                wo = wop.tile([P, DC, E], BF)
                nc.sync.dma_start(wo, wo_d.rearrange("o p n -> p o n"))
                for tt in range(NT):
                    ot = oop.tile([P, E], F32, tag="ot")
                    for et in range(2):
                        pp = mm3.tile([P, 512], F32, tag="p3")
                        for kk in range(DC):
                            nc.tensor.matmul(
                                pp,
                                lhsT=oc[:, kk, P * tt : P * (tt + 1)],
                                rhs=wo[:, kk, 512 * et : 512 * (et + 1)],
                                start=(kk == 0),
                                stop=(kk == DC - 1),
                            )
                        nc.scalar.copy(ot[:, 512 * et : 512 * (et + 1)], pp)
                    nc.sync.dma_start(out_d[P * tt : P * (tt + 1), :], ot)

    nc.compile()
    nc.m = get_hw_module(nc.m)
    return nc


def _prep_inputs(input, Wq, Wk, Wv, Wo):
    """Host-side shard prep: transpose/de-interleave/cast. Returns 8 in_maps."""
    perm = np.concatenate([np.arange(0, E, 2), np.arange(1, E, 2)])

    u = np.arange(E // 2, dtype=np.float64)
    thetas = 10000.0 ** (-2.0 * u / E)
    ang = np.arange(T, dtype=np.float64)[:, None] * thetas[None, :]
    sinh = np.sin(ang).T.reshape(4, P, T).astype(bf16)
    cosh = np.cos(ang).T.reshape(4, P, T).astype(bf16)

    masks = np.zeros((4, P, 512), np.float32)
    f = np.arange(512)
    for r in range(4):
        for p in range(P):
            masks[r, p] = (f >= P * r + p).astype(np.float32)
    masks = masks.astype(bf16)

    xt = [
        np.ascontiguousarray(input[b].T[perm]).reshape(EC, P, T).astype(bf16)
        for b in range(B)
    ]
    WqT, WkT, WvT = Wq.T[perm], Wk.T[perm], Wv.T[perm]
    wq_g = [
        np.ascontiguousarray(WqT[:, DL * g : DL * (g + 1)])
        .reshape(EC, P, DL)
        .astype(bf16)
        for g in range(G)
    ]
    wk_g = [
        np.ascontiguousarray(WkT[:, DL * g : DL * (g + 1)])
        .reshape(EC, P, DL)
        .astype(bf16)
        for g in range(G)
    ]
    wv_g = [
        np.ascontiguousarray(WvT[:, DL * g : DL * (g + 1)])
        .reshape(EC, P, DL)
        .astype(bf16)
        for g in range(G)
    ]
    wo_g = [
        np.ascontiguousarray(Wo.T[DL * g : DL * (g + 1)])
        .reshape(DC, P, E)
        .astype(bf16)
        for g in range(G)
    ]

    in_maps = []
    for c in range(8):
        b, g = c // 2, c % 2
        in_maps.append(
            {
                "xt": xt[b],
                "sinh": sinh,
                "cosh": cosh,
                "wq": wq_g[g],
                "wk": wk_g[g],
                "wv": wv_g[g],
                "wo": wo_g[g],
                "masks": masks,
            }
        )
    return in_maps


def kernel(input, Wq, Wk, Wv, Wo, bo):
    global LAST_RESULT
    input = np.asarray(input, np.float32)
    Wq, Wk, Wv, Wo = (np.asarray(w, np.float32) for w in (Wq, Wk, Wv, Wo))
    bo = np.asarray(bo, np.float32)

    if "nc" not in _CACHE:
        _CACHE["nc"] = _build()
    nc = _CACHE["nc"]

    in_maps = _prep_inputs(input, Wq, Wk, Wv, Wo)
    res = bass_utils.run_bass_kernel_spmd(nc, in_maps, core_ids=list(range(8)))
    LAST_RESULT = res

    out = np.empty((B, T, E), np.float32)
    for b in range(B):
        out[b] = res.results[2 * b]["out"] + res.results[2 * b + 1]["out"] + bo
    return out
